# revision 64
# baseline (speedup 1.0000x reference)
"""Trainium2 Bass kernel for nn_ClawMatrix (cross-modal claw-matrix alignment).

reference per batch element b:
    vp = relu(LN(v @ vW))                [S, D]
    lp = relu(LN(l @ lW))                [S, D]
    sim = vp @ lp.T ; x = c * sim        c = mean(claw)/0.07  (~ -5.8e-4)
    A = softmax(x, axis=-1)
    out = relu(LN([A @ vp, A.T @ lp] @ oW))

Key algebraic identity: the logits x are tiny (|x| < 0.11), and softmax is
shift-invariant so only the per-row deviation y = x - rowmean(x) matters
(|y| <~ 0.06).  Linearizing exp(y) ~= 1 + y makes the softmax row-sum
EXACTLY S, so

    A[s,t] = (1 + c*sim[s,t] - m_s) / S,   m_s = (c/S) * vp_s . lcol

(lcol = column sums of lp).  Both S x S bmms and sim itself then collapse
into D x D matmuls via G = lp^T vp:

    out_pre * S = CONST(row) + c*vp@(G@oW_t - (1/S) lcol x u) + c*lp@(G^T@oW_b)
    CONST = u + lcol@oW_b - (c/S) lcol@(G^T@oW_b),   u = vcol@oW_t

The final LN is scale-invariant, so the 1/S and 256x fp8 scales are never
divided out.  Numerically validated vs the fp64 reference: linearization
error 3e-6; full pipeline ~5e-3 on hardware (budget 2e-2).

Device strategy (one batch element per core, 8 cores, no collectives;
baseline 372.8us -> ~240us):
  - projections, Gt/Gb folds and the final [S,2D]@[2D,D] matmul run
    fp8e4m3 with perf_mode=DoubleRow (k-pair 3D APs, stride%16==0); G runs
    bf16 straight from the projection slabs (no extra fp8 copies)
  - LN: a single scalar Copy evacuates each psum tile to bf16 within
    ~0.6us (PSUM slots are the PE's runway -- keeping the PE continuously
    busy matters doubly on TRN2 because the tensor clock p-state ramps
    0.65 -> 1.2 -> 2.4 GHz and resets on every idle gap); mean/var via
    vector bn_stats/bn_aggr on the bf16 copy (projections) or straight
    from psum (final, exact fp32); relu fused into ScalarE activations
    for the final phase and VectorE mult-sub+max passes for the
    projections
  - vp/lp transposed via XBAR in bf16 (2-byte elements only) then cast to
    fp8; transposes alternate between the two HWDGE rings (sync/scalar)
    because a ring's semaphore wait occupies its host engine's sequencer
  - column sums via PE ones-matvecs; rows -> columns via k=1 matmuls
    against a single one (XBAR cannot transpose 1-row tiles)
  - CONST row stored as bf16 hi+lo on two partitions (stacked by a one-off
    DMA) so one k=2 matmul applies it at ~16-bit precision
  - output written bf16 (host casts to f32); engine work balanced so
    VectorE/ScalarE/both DMA rings each stay under the PE pair cadence
"""

import os
import sys
import numpy as np

for _p in ("/opt/trn_rl_repo", "/root/.axon_site/_ro/trn_rl_repo"):
    if os.path.isdir(_p) and _p not in sys.path:
        sys.path.insert(0, _p)

import ml_dtypes  # noqa: E402

BF16 = ml_dtypes.bfloat16
F8E4 = ml_dtypes.float8_e4m3

P = 128           # partitions
B = 8             # batch / cores
S = 2048          # sequence
D = 768           # feature dim
DPAD = 16         # weight pad: [mean col | zeros]; keeps DR stride %16==0
DW = D + DPAD     # 784
MS = 256.0        # mean-column scale (fp8 subnormal guard)
K8 = 256.0        # fp8 storage scale for Gt/Gb (and CONST, final psum)
EPS = 1e-5
TEMPERATURE = 0.07

_BUILD_CACHE = {}


class _Pool:
    """Manually managed tile-pool lifetime (enter now, exit at any point)."""

    def __init__(self, tc, **kw):
        self._cm = tc.tile_pool(**kw)
        self.pool = self._cm.__enter__()
        self._open = True

    def tile(self, *a, **kw):
        if "name" not in kw:
            kw["name"] = kw.get("tag") or f"t{id(self) % 9973}"
        return self.pool.tile(*a, **kw)

    def close(self):
        if self._open:
            self._cm.__exit__(None, None, None)
            self._open = False


def _build(c_scale: float, trivial: bool, s: int = S, d: int = D):
    """Builds the single-core Bass program. Returns the compiled Bacc module."""
    import concourse.bass as bass
    import concourse.tile as tile
    from concourse import bacc, mybir

    f32 = mybir.dt.float32
    bf16 = mybir.dt.bfloat16
    f8 = mybir.dt.float8e4
    DR = mybir.MatmulPerfMode.DoubleRow
    AF = mybir.ActivationFunctionType
    AX = mybir.AxisListType
    OP = mybir.AluOpType

    st_n = s // P          # 16 row tiles over S
    dt_n = d // P          # 6 tiles over D
    dw = d + DPAD
    d_chunks = [(0, 512), (512, dw - 512)]     # psum-bank chunks over DW
    g_chunks = [(0, 512), (512, d - 512)]
    cc = float(c_scale)

    nc = bacc.Bacc(
        "TRN2",
        target_bir_lowering=False,
        debug=False,
        enable_asserts=False,
        num_devices=B,
    )

    vlT_d = nc.dram_tensor("vlT8", [2 * d, s], f8, kind="ExternalInput")
    vW_d = nc.dram_tensor("vW8", [d, dw], f8, kind="ExternalInput")
    lW_d = nc.dram_tensor("lW8", [d, dw], f8, kind="ExternalInput")
    oW8_d = nc.dram_tensor("oW8", [2 * d, dw], f8, kind="ExternalInput")
    oWb_d = nc.dram_tensor("oWbf", [2 * d, dw], bf16, kind="ExternalInput")
    if not trivial:
        vb_d = nc.dram_tensor("vb", [1, dw], bf16, kind="ExternalInput")
        lb_d = nc.dram_tensor("lb", [1, dw], bf16, kind="ExternalInput")
        ob_d = nc.dram_tensor("ob", [1, dw], bf16, kind="ExternalInput")
        vg_d = nc.dram_tensor("vg", [1, d], f32, kind="ExternalInput")
        vbe_d = nc.dram_tensor("vbeta", [1, d], f32, kind="ExternalInput")
        lg_d = nc.dram_tensor("lg", [1, d], f32, kind="ExternalInput")
        lbe_d = nc.dram_tensor("lbeta", [1, d], f32, kind="ExternalInput")
        og_d = nc.dram_tensor("og", [1, d], f32, kind="ExternalInput")
        obe_d = nc.dram_tensor("obeta", [1, d], f32, kind="ExternalInput")
    out_d = nc.dram_tensor("out", [s, d], bf16, kind="ExternalOutput")

    with tile.TileContext(nc) as tc:
        pp = _Pool(tc, name="persist", bufs=1)
        sp = _Pool(tc, name="small", bufs=4)
        tmp_p = _Pool(tc, name="tmp", bufs=3)
        pf_p = _Pool(tc, name="pf", bufs=4)

        eps_sb = pp.tile([P, 1], f32, tag='eps')
        nc.vector.memset(eps_sb[:], EPS)
        ones2 = pp.tile([2, s], bf16, tag='ones2')
        nc.vector.memset(ones2[:], 1.0)
        onescol = pp.tile([P, 1], bf16, tag='onescol')
        nc.vector.memset(onescol[:], 1.0)

        if not trivial:
            ones_sb = pp.tile([1, P], bf16, tag="ones_sb")
            nc.vector.memset(ones_sb[:], 1.0)
            b_sb = {}
            aff = {}
            for nm, dd in (("vb", vb_d), ("lb", lb_d), ("ob", ob_d)):
                t = pp.tile([1, dw], bf16, tag=nm)
                nc.sync.dma_start(out=t[:], in_=dd.ap())
                b_sb[nm] = t
            for nm, dd in (("vg", vg_d), ("vbeta", vbe_d), ("lg", lg_d),
                           ("lbeta", lbe_d), ("og", og_d), ("obeta", obe_d)):
                t = pp.tile([P, d], f32, tag=nm)
                src = bass.AP(tensor=dd.ap().tensor, offset=0,
                              ap=[[0, P], [1, d]])
                nc.sync.dma_start(out=t[:], in_=src)
                aff[nm] = t

        # column/row statistics tiles ([P, 6, 1] column layout; the rows are
        # PE-transposed into columns via k=1 matmuls against ones1b)
        vcolK = pp.tile([P, dt_n, 1], bf16, tag='vcolK')   # 256*vcol
        lcolK = pp.tile([P, dt_n, 1], bf16, tag='lcolK')   # 256*lcol
        lcoln = pp.tile([P, dt_n, 1], bf16, tag='lcoln')   # -lcol/S
        vcol_row = pp.tile([1, d], bf16, tag='vcol_row')
        lcol_row = pp.tile([1, d], bf16, tag='lcol_row')
        ones1b = pp.tile([1, 1], bf16, tag='ones1b')
        nc.vector.memset(ones1b[:], 1.0)
        uneg_row = pp.tile([1, dw], bf16, tag='uneg_row')  # -u/(2S)
        chi_r = pp.tile([1, dw], bf16, tag='chi_r')        # CONST hi row
        clo_r = pp.tile([1, dw], bf16, tag='clo_r')        # CONST lo row
        chilo2 = pp.tile([2, dw], bf16, tag='chilo2')      # stacked hi/lo

        # big SBUF slabs.  Pool closes must be LIFO per memory space, so the
        # transient projection-era pools (in8/w8/xbf/xTbf) are created LAST
        # and closed mid-program in reverse order, freeing their region for
        # the g_p pool pushed after them.
        xT8_p = _Pool(tc, name="xT8", bufs=1)
        ow8_p = _Pool(tc, name="ow8", bufs=1)
        owb_p = _Pool(tc, name="owbf", bufs=1)
        ps_p = _Pool(tc, name="psuni", bufs=4, space=bass.MemorySpace.PSUM)
        xbf_p = _Pool(tc, name="xbf", bufs=2)      # lp and vp slabs (bf16)
        xTbf_p = _Pool(tc, name="xTbf", bufs=1)    # cycled: lpT then vpT
        in_p = _Pool(tc, name="in8", bufs=1)
        w8_p = _Pool(tc, name="w8", bufs=2)

        def psum_tile(n):
            t = ps_p.tile([P, d], f32, tag="ps")
            return t[:, :n]

        vlT_sb = in_p.tile([P, 2 * dt_n, s], f8, tag="in8", name="vlT_sb")
        # all input loads up front on the sync ring, l k-tiles first
        # (projection order is l then v), first halves before second halves
        for h in range(2):
            for j in list(range(dt_n, 2 * dt_n)) + list(range(dt_n)):
                nc.sync.dma_start(
                    out=vlT_sb[:, j, h * (s // 2):(h + 1) * (s // 2)],
                    in_=vlT_d.ap()[j * P:(j + 1) * P,
                                   h * (s // 2):(h + 1) * (s // 2)])

        vpT8 = xT8_p.tile([P, dt_n, s], f8, tag='vpT8', name='vpT8')
        lpT8 = xT8_p.tile([P, dt_n, s], f8, tag='lpT8', name='lpT8')

        oW8_sb = ow8_p.tile([P, 2 * dt_n, dw], f8, tag="ow8", name="oW8_sb")
        oWb_sb = owb_p.tile([P, 2 * dt_n, dw], bf16, tag="owbf",
                            name="oWb_sb")

        def layernorm_relu_pair(pss, dsts, g_nm, be_nm, tagsfx,
                                dst_f32=False, use_pf=False):
            """LN+relu over [:, :d] of a pair of psum tiles.

            Row mean/var come from bn_stats+bn_aggr (one VectorE data pass
            per 384-wide group, exact fp32) -- no separate mean/Square
            passes.  The out passes read the psum directly.
            Scale-invariant: psum may be any uniform multiple of the true
            pre-LN activations (EPS is negligible vs var in both scalings).
            """
            n = len(pss)
            mv2 = sp.tile([P, 2, 2], f32, tag="mv" + tagsfx)
            pfs = []
            if use_pf and trivial:
                # proj path: one scalar Copy evacuates the psum within
                # ~0.6us; bn_stats runs on the bf16 copy (fast SBUF reads)
                for i, ps in enumerate(pss):
                    pf = pf_p.tile([P, d], bf16, tag="pf")
                    nc.scalar.activation(out=pf[:], in_=ps[:, :d],
                                         func=AF.Copy)
                    st12 = sp.tile([P, 2, 6], f32, tag="sb%d" % i + tagsfx)
                    nc.vector.bn_stats(out=st12[:, 0, :], in_=pf[:, 0:512])
                    nc.vector.bn_stats(out=st12[:, 1, :], in_=pf[:, 512:768])
                    nc.vector.bn_aggr(out=mv2[:, i, :], in_=st12[:])
                    pfs.append(pf)
            else:
                for i, ps in enumerate(pss):
                    st12 = sp.tile([P, 2, 6], f32, tag="st%d" % i + tagsfx)
                    nc.vector.bn_stats(out=st12[:, 0, :], in_=ps[:, 0:512])
                    nc.vector.bn_stats(out=st12[:, 1, :], in_=ps[:, 512:768])
                    nc.vector.bn_aggr(out=mv2[:, i, :], in_=st12[:])
            rstd = sp.tile([P, 2], f32, tag="rstd" + tagsfx)
            nc.scalar.activation(out=rstd[:, :n], in_=mv2[:, :n, 1],
                                 func=AF.Sqrt, bias=eps_sb[:])
            nc.vector.reciprocal(out=rstd[:, :n], in_=rstd[:, :n])
            mr = sp.tile([P, 2], f32, tag="mr" + tagsfx)
            nc.vector.tensor_tensor(out=mr[:, :n], in0=mv2[:, :n, 0],
                                    in1=rstd[:, :n], op=OP.mult)
            nmr = sp.tile([P, 2], f32, tag="nmr" + tagsfx)
            nc.vector.tensor_scalar(out=nmr[:, :n], in0=mr[:, :n],
                                    scalar1=-1.0, scalar2=None, op0=OP.mult)
            for i, (ps, dst) in enumerate(zip(pss, dsts)):
                if trivial:
                    if use_pf:
                        tmp = tmp_p.tile([P, d], bf16, tag="tmp")
                        nc.vector.tensor_scalar(out=tmp[:], in0=pfs[i][:],
                                                scalar1=rstd[:, i:i + 1],
                                                scalar2=mr[:, i:i + 1],
                                                op0=OP.mult,
                                                op1=OP.subtract)
                        nc.vector.tensor_scalar_max(out=dst, in0=tmp[:],
                                                    scalar1=0.0)
                        continue
                    # fused relu((y - m) * rstd) on ScalarE; VectorE keeps
                    # only the stats so both engines stay under the PE rate
                    nc.scalar.activation(out=dst, in_=ps[:, :d],
                                         func=AF.Relu,
                                         bias=nmr[:, i:i + 1],
                                         scale=rstd[:, i:i + 1])
                else:
                    nrm = sp.tile([P, d], f32, tag="nrm" + tagsfx, bufs=2)
                    nc.vector.tensor_scalar(out=nrm[:], in0=ps[:, :d],
                                            scalar1=rstd[:, i:i + 1],
                                            scalar2=mr[:, i:i + 1],
                                            op0=OP.mult, op1=OP.subtract)
                    nc.vector.tensor_mul(out=nrm[:], in0=nrm[:],
                                         in1=aff[g_nm][:])
                    nc.vector.tensor_add(out=nrm[:], in0=nrm[:],
                                         in1=aff[be_nm][:])
                    nc.vector.tensor_scalar_max(out=dst, in0=nrm[:],
                                                scalar1=0.0)

        # ---------- phase A/B: projections (fp8 DoubleRow) ----------
        def load_w(W_d):
            W_sb = w8_p.tile([P, dt_n, dw], f8, tag="w8", name="W_sb")
            for j in range(dt_n):
                nc.gpsimd.dma_start(out=W_sb[:, j, :],
                                    in_=W_d.ap()[j * P:(j + 1) * P, :])
            return W_sb

        def proj(base, W_sb, xpT8_dst, bias_nm, g_nm, be_nm,
                 defer_casts=False):
            xp_bf = xbf_p.tile([P, st_n, d], bf16, tag='xbf', name='xp_bf')
            xpT_bf = xTbf_p.tile([P, dt_n, s], bf16, tag='xTbf',
                                 name='xpT_bf')
            for st0 in range(0, st_n, 2):
                pss, dsts = [], []
                for st in (st0, st0 + 1):
                    ps = psum_tile(d)
                    for c0, cl in g_chunks:
                        for q in range(dt_n // 2):
                            nc.tensor.matmul(
                                ps[:, c0:c0 + cl],
                                vlT_sb[:, base + 2 * q:base + 2 * q + 2,
                                       st * P:(st + 1) * P],
                                W_sb[:, 2 * q:2 * q + 2, c0:c0 + cl],
                                start=(q == 0),
                                stop=(q == dt_n // 2 - 1 and trivial),
                                perf_mode=DR)
                        if not trivial:
                            nc.tensor.matmul(
                                ps[:, c0:c0 + cl], ones_sb[:1, :],
                                b_sb[bias_nm][:1, c0:c0 + cl],
                                start=False, stop=True)
                    pss.append(ps)
                    dsts.append(xp_bf[:, st, :])
                layernorm_relu_pair(pss, dsts, g_nm, be_nm, "p",
                                    use_pf=True)
                for st in (st0, st0 + 1):
                    ring = nc.scalar if st % 2 == 0 else nc.sync
                    ring.dma_start_transpose(
                        out=xpT_bf[:, :, st * P:(st + 1) * P],
                        in_=xp_bf[:, st, :])
            if not defer_casts:
                for j in range(dt_n):
                    nc.vector.tensor_copy(out=xpT8_dst[:, j, :],
                                          in_=xpT_bf[:, j, :])
            return xp_bf, xpT_bf

        def colsum_row(x_bf, row_dst):
            # column sums as a [1, d] row via a PE ones-matvec (keeps the
            # big reduces off VectorE)
            ps = psum_tile(d)
            for c0, cl in g_chunks:
                for q in range(st_n):
                    nc.tensor.matmul(ps[:1, c0:c0 + cl], onescol[:, :1],
                                     x_bf[:, q, c0:c0 + cl],
                                     start=(q == 0), stop=(q == st_n - 1))
            nc.scalar.activation(out=row_dst[:1, :], in_=ps[:1, :d],
                                 func=AF.Copy)

        def row_to_cols(row, col_specs):
            # PE-transpose a [1, d] row into [128, 1] columns: k=1 matmul of
            # the row slice against a single one, then scaled PSUM reads
            for j in range(dt_n):
                ps = psum_tile(16)
                nc.tensor.matmul(ps[:, :1], row[:1, j * P:(j + 1) * P],
                                 ones1b[:1, :1], start=True, stop=True)
                for dst, scl in col_specs:
                    nc.vector.tensor_scalar(out=dst[:, j, :], in0=ps[:, :1],
                                            scalar1=scl, scalar2=None,
                                            op0=OP.mult)

        Wl_sb = load_w(lW_d)
        Wv_sb = load_w(vW_d)
        lp_bf, lpT_bf = proj(dt_n, Wl_sb, lpT8, "lb", "lg", "lbeta",
                             defer_casts=True)
        # lp casts on the gpsimd queue (only these 6 ride it, after the
        # preloaded weights) so the proj-l tail's VectorE stays clear
        for j in range(dt_n):
            nc.gpsimd.tensor_copy(out=lpT8[:, j, :], in_=lpT_bf[:, j, :])
        vp_bf, vpT_bf = proj(0, Wv_sb, vpT8, "vb", "vg", "vbeta",
                             defer_casts=True)
        # colsums after proj-v so its matmuls aren't queued behind them on
        # the in-order PE; the lp colsum also fills the vp-LN drain
        colsum_row(lp_bf, lcol_row)
        row_to_cols(lcol_row, [(lcolK, MS), (lcoln, -1.0 / s)])
        colsum_row(vp_bf, vcol_row)
        row_to_cols(vcol_row, [(vcolK, MS)])
        # oW loads on the gpsimd ring (sync carries half the transposes)
        for j in range(2 * dt_n):
            nc.gpsimd.dma_start(out=oW8_sb[:, j, :],
                                in_=oW8_d.ap()[j * P:(j + 1) * P, :])
            nc.gpsimd.dma_start(out=oWb_sb[:, j, :],
                                in_=oWb_d.ap()[j * P:(j + 1) * P, :])
        # input/weight slabs are dead now (xbf/xTbf stay: G and the
        # deferred vpT8 casts read them)
        w8_p.close()
        in_p.close()

        # ---------- phase C: G = lp^T vp (fp8 DR), transpose, folds --------
        g_p = _Pool(tc, name="gp", bufs=1)
        ot_p = _Pool(tc, name="outsb", bufs=2)
        G8 = g_p.tile([P, dt_n, d], f8, tag='G8', name='G8')
        G_bf = g_p.tile([P, dt_n, d], bf16, tag='G_bf', name='G_bf')
        GT_bf = g_p.tile([P, dt_n, d], bf16, tag='GT_bf', name='GT_bf')
        GT8 = g_p.tile([P, dt_n, d], f8, tag='GT8', name='GT8')
        Gt8 = g_p.tile([P, dt_n, dw], f8, tag='Gt8', name='Gt8')
        Gb8 = g_p.tile([P, dt_n, dw], f8, tag='Gb8', name='Gb8')
        Gb_bf = g_p.tile([P, dt_n, dw], bf16, tag='Gb_bf', name='Gb_bf')

        for j in range(dt_n):
            ps = psum_tile(d)
            for c0, cl in g_chunks:
                for q in range(st_n):
                    nc.tensor.matmul(
                        ps[:, c0:c0 + cl],
                        lp_bf[:, q, j * P:(j + 1) * P],
                        vp_bf[:, q, c0:c0 + cl],
                        start=(q == 0), stop=(q == st_n - 1))
            # G8 at half scale (G max ~452 vs fp8 max 448); G_bf exact
            nc.scalar.activation(out=G8[:, j, :], in_=ps[:, :d],
                                 func=AF.Copy, scale=0.5)
            nc.vector.tensor_copy(out=G_bf[:, j, :], in_=ps[:, :d])
            # deferred vpT8 cast rides the G phase's vector slack instead of
            # blocking the proj-v LN tail
            nc.vector.tensor_copy(out=vpT8[:, j, :], in_=vpT_bf[:, j, :])
        for j in range(dt_n):
            nc.scalar.dma_start_transpose(
                out=GT_bf[:, :, j * P:(j + 1) * P], in_=G_bf[:, j, :])
        for j in range(dt_n):
            nc.vector.tensor_scalar(out=GT8[:, j, :], in0=GT_bf[:, j, :],
                                    scalar1=0.5, scalar2=None, op0=OP.mult)

        # u = 256*vcol @ oW_t ; store uneg_row = -u/(2S) (rank-1 operand at
        # the G8 half-scale so one evacuation scale covers the Gt psum).
        # Placed after G so the vcol XBAR/cast latency hides under it.
        ps = psum_tile(d)
        for c0, cl in g_chunks:
            for k in range(dt_n):
                nc.tensor.matmul(ps[:1, c0:c0 + cl], vcolK[:, k, :],
                                 oWb_sb[:, k, c0:c0 + cl],
                                 start=(k == 0), stop=(k == dt_n - 1))
        nc.scalar.activation(out=uneg_row[:1, :d], in_=ps[:1, :d],
                             func=AF.Copy, scale=-1.0 / (2.0 * MS * s))

        # Gb = G^T @ oW_b  (stored fp8 at 2*K8*c, plus bf16 copy for CONST)
        for j in range(dt_n):
            ps = psum_tile(d)
            for c0, cl in g_chunks:
                for q in range(dt_n // 2):
                    nc.tensor.matmul(
                        ps[:, c0:c0 + cl],
                        G8[:, 2 * q:2 * q + 2, j * P:(j + 1) * P],
                        oW8_sb[:, dt_n + 2 * q:dt_n + 2 * q + 2, c0:c0 + cl],
                        start=(q == 0), stop=(q == dt_n // 2 - 1),
                        perf_mode=DR)
            nc.scalar.activation(out=Gb8[:, j, :d], in_=ps[:, :d],
                                 func=AF.Copy, scale=2.0 * K8 * cc)
            nc.vector.tensor_scalar(out=Gb_bf[:, j, :d], in0=ps[:, :d],
                                    scalar1=2.0 * K8 * cc, scalar2=None,
                                    op0=OP.mult)

        # Gt = G @ oW_t - (1/S) lcol x u   (rank-1 via k=1 bf16 matmul)
        for j in range(dt_n):
            ps = psum_tile(d)
            for c0, cl in g_chunks:
                for q in range(dt_n // 2):
                    nc.tensor.matmul(
                        ps[:, c0:c0 + cl],
                        GT8[:, 2 * q:2 * q + 2, j * P:(j + 1) * P],
                        oW8_sb[:, 2 * q:2 * q + 2, c0:c0 + cl],
                        start=(q == 0), stop=False,
                        perf_mode=DR)
                nc.tensor.matmul(
                    ps[:, c0:c0 + cl],
                    lcol_row[:1, j * P:(j + 1) * P],
                    uneg_row[:1, c0:c0 + cl],
                    start=False, stop=True)
            nc.scalar.activation(out=Gt8[:, j, :d], in_=ps[:, :d],
                                 func=AF.Copy, scale=2.0 * K8 * cc)

        # CONST row (256x scale): 256*vcol@oW_t + 256*lcol@oW_b
        #                         - (1/S) lcol@(2*K8*c*Gb) ; + 256*S*ob
        if not trivial:
            obS = sp.tile([1, dw], bf16, tag="obS")
            nc.vector.tensor_scalar(out=obS[:], in0=b_sb["ob"][:],
                                    scalar1=MS * float(s), scalar2=None,
                                    op0=OP.mult)
        ps = psum_tile(d)
        for c0, cl in g_chunks:
            for k in range(dt_n):
                nc.tensor.matmul(ps[:1, c0:c0 + cl], vcolK[:, k, :],
                                 oWb_sb[:, k, c0:c0 + cl],
                                 start=(k == 0), stop=False)
            for k in range(dt_n):
                nc.tensor.matmul(ps[:1, c0:c0 + cl], lcolK[:, k, :],
                                 oWb_sb[:, dt_n + k, c0:c0 + cl],
                                 start=False, stop=False)
            for k in range(dt_n):
                nc.tensor.matmul(ps[:1, c0:c0 + cl], lcoln[:, k, :],
                                 Gb_bf[:, k, c0:c0 + cl],
                                 start=False,
                                 stop=(trivial and k == dt_n - 1))
            if not trivial:
                nc.tensor.matmul(ps[:1, c0:c0 + cl], ones_sb[:1, :1],
                                 obS[:1, c0:c0 + cl], start=False, stop=True)
        nc.scalar.activation(out=chi_r[:1, :d], in_=ps[:1, :d],
                             func=AF.Copy)
        nc.vector.tensor_tensor(out=clo_r[:1, :d], in0=ps[:1, :d],
                                in1=chi_r[:1, :d], op=OP.subtract)
        nc.gpsimd.dma_start(out=chilo2[0:1, :d], in_=chi_r[:1, :d])
        nc.gpsimd.dma_start(out=chilo2[1:2, :d], in_=clo_r[:1, :d])

        # ---------- phase D: final  psum = vp@Gt + lp@Gb + 1 x (hi+lo) ------
        for st0 in range(0, st_n, 2):
            pss, ots = [], []
            for st in (st0, st0 + 1):
                ps = psum_tile(d)
                for c0, cl in g_chunks:
                    for q in range(dt_n // 2):
                        nc.tensor.matmul(
                            ps[:, c0:c0 + cl],
                            vpT8[:, 2 * q:2 * q + 2, st * P:(st + 1) * P],
                            Gt8[:, 2 * q:2 * q + 2, c0:c0 + cl],
                            start=(q == 0), stop=False, perf_mode=DR)
                    for q in range(dt_n // 2):
                        nc.tensor.matmul(
                            ps[:, c0:c0 + cl],
                            lpT8[:, 2 * q:2 * q + 2, st * P:(st + 1) * P],
                            Gb8[:, 2 * q:2 * q + 2, c0:c0 + cl],
                            start=False, stop=False, perf_mode=DR)
                    nc.tensor.matmul(
                        ps[:, c0:c0 + cl],
                        ones2[:2, st * P:(st + 1) * P],
                        chilo2[:2, c0:c0 + cl],
                        start=False, stop=True)
                pss.append(ps)
                ots.append(ot_p.tile([P, d], bf16, tag="ot", bufs=3))
            layernorm_relu_pair(pss, [o[:] for o in ots],
                                "og", "obeta", "o", dst_f32=True)
            for i, st in enumerate((st0, st0 + 1)):
                nc.sync.dma_start(
                    out=out_d.ap()[st * P:(st + 1) * P, :], in_=ots[i][:])

        ot_p.close()
        g_p.close()
        xTbf_p.close()
        xbf_p.close()
        ps_p.close()
        owb_p.close()
        ow8_p.close()
        xT8_p.close()
        pf_p.close()
        tmp_p.close()
        sp.close()
        pp.close()
        # (ps_p is on the separate PSUM stack; SBUF closes above are LIFO)

    nc.compile()
    return nc


def _get_program(c_scale: float, trivial: bool, s: int = S, d: int = D):
    key = (round(float(c_scale), 12), trivial, s, d)
    if key not in _BUILD_CACHE:
        _BUILD_CACHE[key] = _build(c_scale, trivial, s, d)
    return _BUILD_CACHE[key]


def _w_ext(W, dtype):
    """[K, N] weights -> [K, N + DPAD] with col N = MS * row-mean, pad 0."""
    W = np.asarray(W, np.float32)
    k = W.shape[0]
    ext = np.zeros((k, W.shape[1] + DPAD), np.float32)
    ext[:, :W.shape[1]] = W
    ext[:, W.shape[1]] = MS * W.mean(axis=1)
    return np.ascontiguousarray(ext.astype(dtype))


def _prep_in_maps(vision, language, vW, lW, oW, trivial, extras):
    n_b = vision.shape[0]
    vW8 = _w_ext(vW, F8E4)
    lW8 = _w_ext(lW, F8E4)
    oW_ext = _w_ext(oW, np.float32)
    oW_ext[:, D] = 0.0            # final LN mean is computed on device
    oW8 = np.ascontiguousarray(oW_ext.astype(F8E4))
    oWbf = np.ascontiguousarray(oW_ext.astype(BF16))
    in_maps = []
    for b in range(n_b):
        vlT = np.concatenate([vision[b].T, language[b].T], 0)
        m = {
            "vlT8": np.ascontiguousarray(vlT.astype(F8E4)),
            "vW8": vW8, "lW8": lW8, "oW8": oW8, "oWbf": oWbf,
        }
        if not trivial:
            m.update(extras)
        in_maps.append(m)
    return in_maps


def _program_and_inmaps(inputs):
    """(compiled program, per-core input maps) for the given full inputs."""
    vision = np.asarray(inputs["vision_features"], np.float32)
    language = np.asarray(inputs["language_features"], np.float32)
    c_scale = float(np.asarray(inputs["claw"], np.float32).mean()) / TEMPERATURE
    nc = _get_program(c_scale, True)
    in_maps = _prep_in_maps(vision, language, inputs["vW"], inputs["lW"],
                            inputs["oW"], True, {})
    return nc, in_maps


def kernel(vision_features, language_features, vW, vb, vg, vbeta,
           lW, lb, lg, lbeta, claw, oW, ob, og, obeta):
    from concourse import bass_utils

    vision = np.asarray(vision_features, np.float32)
    language = np.asarray(language_features, np.float32)
    c_scale = float(np.asarray(claw, np.float32).mean()) / TEMPERATURE
    # linearized softmax: valid when the logit scale is small (|y| << 1);
    # |sim| <= 1.5*D is a conservative row-norm bound
    assert abs(c_scale) * 1.5 * D < 0.8, "logit scale too large to linearize"

    trivial = (
        np.all(np.asarray(vb) == 0) and np.all(np.asarray(lb) == 0)
        and np.all(np.asarray(ob) == 0)
        and np.all(np.asarray(vg) == 1) and np.all(np.asarray(vbeta) == 0)
        and np.all(np.asarray(lg) == 1) and np.all(np.asarray(lbeta) == 0)
        and np.all(np.asarray(og) == 1) and np.all(np.asarray(obeta) == 0)
    )

    def bias_ext(bv):
        bv = np.asarray(bv, np.float32).reshape(D)
        ext = np.zeros(D + DPAD, np.float32)
        ext[:D] = bv
        ext[D] = MS * bv.mean()
        return ext.reshape(1, D + DPAD).astype(BF16)

    extras = {}
    if not trivial:
        extras = {
            "vb": bias_ext(vb), "lb": bias_ext(lb), "ob": bias_ext(ob),
            "vg": np.asarray(vg, np.float32).reshape(1, D),
            "vbeta": np.asarray(vbeta, np.float32).reshape(1, D),
            "lg": np.asarray(lg, np.float32).reshape(1, D),
            "lbeta": np.asarray(lbeta, np.float32).reshape(1, D),
            "og": np.asarray(og, np.float32).reshape(1, D),
            "obeta": np.asarray(obeta, np.float32).reshape(1, D),
        }

    nc = _get_program(c_scale, trivial)
    in_maps = _prep_in_maps(vision, language, vW, lW, oW, trivial, extras)
    res = bass_utils.run_bass_kernel_spmd(nc, in_maps,
                                          core_ids=list(range(B)))
    return np.stack([np.asarray(res.results[b]["out"], np.float32)
                     for b in range(B)], axis=0)


# revision 65
# speedup vs baseline: 1.1520x; 1.1520x over previous
"""Trainium2 Bass kernel for nn_ClawMatrix (cross-modal claw-matrix alignment).

reference per batch element b:
    vp = relu(LN(v @ vW))                [S, D]
    lp = relu(LN(l @ lW))                [S, D]
    sim = vp @ lp.T ; x = c * sim        c = mean(claw)/0.07  (~ -5.8e-4)
    A = softmax(x, axis=-1)
    out = relu(LN([A @ vp, A.T @ lp] @ oW))

Key algebraic identity: the logits x are tiny (|x| < 0.11), and softmax is
shift-invariant so only the per-row deviation y = x - rowmean(x) matters
(|y| <~ 0.06).  Linearizing exp(y) ~= 1 + y makes the softmax row-sum
EXACTLY S, so

    A[s,t] = (1 + c*sim[s,t] - m_s) / S,   m_s = (c/S) * vp_s . lcol

(lcol = column sums of lp).  Both S x S bmms and sim itself then collapse
into D x D matmuls via G = lp^T vp:

    out_pre * S = CONST(row) + c*vp@(G@oW_t - (1/S) lcol x u) + c*lp@(G^T@oW_b)
    CONST = u + lcol@oW_b - (c/S) lcol@(G^T@oW_b),   u = vcol@oW_t

The final LN is scale-invariant, so the 1/S and 256x fp8 scales are never
divided out.  Numerically validated vs the fp64 reference: linearization
error 3e-6; full pipeline ~5e-3 on hardware (budget 2e-2).

Device strategy (one batch element per core, 8 cores, no collectives;
baseline 372.8us -> ~240us):
  - projections, Gt/Gb folds and the final [S,2D]@[2D,D] matmul run
    fp8e4m3 with perf_mode=DoubleRow (k-pair 3D APs, stride%16==0); G runs
    bf16 straight from the projection slabs (no extra fp8 copies)
  - LN: a single scalar Copy evacuates each psum tile to bf16 within
    ~0.6us (PSUM slots are the PE's runway -- keeping the PE continuously
    busy matters doubly on TRN2 because the tensor clock p-state ramps
    0.65 -> 1.2 -> 2.4 GHz and resets on every idle gap); mean/var via
    vector bn_stats/bn_aggr on the bf16 copy (projections) or straight
    from psum (final, exact fp32); relu fused into ScalarE activations
    for the final phase and VectorE mult-sub+max passes for the
    projections
  - vp/lp transposed via XBAR in bf16 (2-byte elements only) then cast to
    fp8; transposes alternate between the two HWDGE rings (sync/scalar)
    because a ring's semaphore wait occupies its host engine's sequencer
  - column sums via PE ones-matvecs; rows -> columns via k=1 matmuls
    against a single one (XBAR cannot transpose 1-row tiles)
  - CONST row stored as bf16 hi+lo on two partitions (stacked by a one-off
    DMA) so one k=2 matmul applies it at ~16-bit precision
  - output written bf16 (host casts to f32); engine work balanced so
    VectorE/ScalarE/both DMA rings each stay under the PE pair cadence
"""

import os
import sys
import numpy as np

for _p in ("/opt/trn_rl_repo", "/root/.axon_site/_ro/trn_rl_repo"):
    if os.path.isdir(_p) and _p not in sys.path:
        sys.path.insert(0, _p)

import ml_dtypes  # noqa: E402

BF16 = ml_dtypes.bfloat16
F8E4 = ml_dtypes.float8_e4m3

P = 128           # partitions
B = 8             # batch / cores
S = 2048          # sequence
D = 768           # feature dim
DPAD = 16         # weight pad: [mean col | zeros]; keeps DR stride %16==0
DW = D + DPAD     # 784
MS = 256.0        # mean-column scale (fp8 subnormal guard)
K8 = 256.0        # fp8 storage scale for Gt/Gb (and CONST, final psum)
EPS = 1e-5
TEMPERATURE = 0.07

_BUILD_CACHE = {}


class _Pool:
    """Manually managed tile-pool lifetime (enter now, exit at any point)."""

    def __init__(self, tc, **kw):
        self._cm = tc.tile_pool(**kw)
        self.pool = self._cm.__enter__()
        self._open = True

    def tile(self, *a, **kw):
        if "name" not in kw:
            kw["name"] = kw.get("tag") or f"t{id(self) % 9973}"
        return self.pool.tile(*a, **kw)

    def close(self):
        if self._open:
            self._cm.__exit__(None, None, None)
            self._open = False


def _build(c_scale: float, trivial: bool, s: int = S, d: int = D):
    """Builds the single-core Bass program. Returns the compiled Bacc module."""
    import concourse.bass as bass
    import concourse.tile as tile
    from concourse import bacc, mybir

    f32 = mybir.dt.float32
    bf16 = mybir.dt.bfloat16
    f8 = mybir.dt.float8e4
    DR = mybir.MatmulPerfMode.DoubleRow
    AF = mybir.ActivationFunctionType
    AX = mybir.AxisListType
    OP = mybir.AluOpType

    st_n = s // P          # 16 row tiles over S
    dt_n = d // P          # 6 tiles over D
    dw = d + DPAD
    d_chunks = [(0, 512), (512, dw - 512)]     # psum-bank chunks over DW
    g_chunks = [(0, 512), (512, d - 512)]
    cc = float(c_scale)

    nc = bacc.Bacc(
        "TRN2",
        target_bir_lowering=False,
        debug=False,
        enable_asserts=False,
        num_devices=B,
    )

    vlT_d = nc.dram_tensor("vlT8", [2 * d, s], f8, kind="ExternalInput")
    vW_d = nc.dram_tensor("vW8", [d, dw], f8, kind="ExternalInput")
    lW_d = nc.dram_tensor("lW8", [d, dw], f8, kind="ExternalInput")
    oW8_d = nc.dram_tensor("oW8", [2 * d, dw], f8, kind="ExternalInput")
    oWb_d = nc.dram_tensor("oWbf", [2 * d, dw], bf16, kind="ExternalInput")
    if not trivial:
        vb_d = nc.dram_tensor("vb", [1, dw], bf16, kind="ExternalInput")
        lb_d = nc.dram_tensor("lb", [1, dw], bf16, kind="ExternalInput")
        ob_d = nc.dram_tensor("ob", [1, dw], bf16, kind="ExternalInput")
        vg_d = nc.dram_tensor("vg", [1, d], f32, kind="ExternalInput")
        vbe_d = nc.dram_tensor("vbeta", [1, d], f32, kind="ExternalInput")
        lg_d = nc.dram_tensor("lg", [1, d], f32, kind="ExternalInput")
        lbe_d = nc.dram_tensor("lbeta", [1, d], f32, kind="ExternalInput")
        og_d = nc.dram_tensor("og", [1, d], f32, kind="ExternalInput")
        obe_d = nc.dram_tensor("obeta", [1, d], f32, kind="ExternalInput")
    out_d = nc.dram_tensor("out", [s, d], bf16, kind="ExternalOutput")

    with tile.TileContext(nc) as tc:
        pp = _Pool(tc, name="persist", bufs=1)
        sp = _Pool(tc, name="small", bufs=4)
        tmp_p = _Pool(tc, name="tmp", bufs=3)
        pf_p = _Pool(tc, name="pf", bufs=4)

        eps_sb = pp.tile([P, 1], f32, tag='eps')
        nc.vector.memset(eps_sb[:], EPS)
        ones2 = pp.tile([2, s], bf16, tag='ones2')
        nc.vector.memset(ones2[:], 1.0)
        onescol = pp.tile([P, 1], bf16, tag='onescol')
        nc.vector.memset(onescol[:], 1.0)

        if not trivial:
            ones_sb = pp.tile([1, P], bf16, tag="ones_sb")
            nc.vector.memset(ones_sb[:], 1.0)
            b_sb = {}
            aff = {}
            for nm, dd in (("vb", vb_d), ("lb", lb_d), ("ob", ob_d)):
                t = pp.tile([1, dw], bf16, tag=nm)
                nc.sync.dma_start(out=t[:], in_=dd.ap())
                b_sb[nm] = t
            for nm, dd in (("vg", vg_d), ("vbeta", vbe_d), ("lg", lg_d),
                           ("lbeta", lbe_d), ("og", og_d), ("obeta", obe_d)):
                t = pp.tile([P, d], f32, tag=nm)
                src = bass.AP(tensor=dd.ap().tensor, offset=0,
                              ap=[[0, P], [1, d]])
                nc.sync.dma_start(out=t[:], in_=src)
                aff[nm] = t

        # column/row statistics tiles ([P, 6, 1] column layout; the rows are
        # PE-transposed into columns via k=1 matmuls against ones1b)
        vcolK = pp.tile([P, dt_n, 1], bf16, tag='vcolK')   # 256*vcol
        lcolK = pp.tile([P, dt_n, 1], bf16, tag='lcolK')   # 256*lcol
        lcoln = pp.tile([P, dt_n, 1], bf16, tag='lcoln')   # -lcol/S
        vcol_row = pp.tile([1, d], bf16, tag='vcol_row')
        lcol_row = pp.tile([1, d], bf16, tag='lcol_row')
        ones1b = pp.tile([1, 1], bf16, tag='ones1b')
        nc.vector.memset(ones1b[:], 1.0)
        uneg_row = pp.tile([1, dw], bf16, tag='uneg_row')  # -u/(2S)
        chi_r = pp.tile([1, dw], bf16, tag='chi_r')        # CONST hi row
        clo_r = pp.tile([1, dw], bf16, tag='clo_r')        # CONST lo row
        chilo2 = pp.tile([2, dw], bf16, tag='chilo2')      # stacked hi/lo

        # big SBUF slabs.  Pool closes must be LIFO per memory space, so the
        # transient projection-era pools (in8/w8/xbf/xTbf) are created LAST
        # and closed mid-program in reverse order, freeing their region for
        # the g_p pool pushed after them.
        xT8_p = _Pool(tc, name="xT8", bufs=1)
        ow8_p = _Pool(tc, name="ow8", bufs=1)
        owb_p = _Pool(tc, name="owbf", bufs=1)
        ps_p = _Pool(tc, name="psuni", bufs=4, space=bass.MemorySpace.PSUM)
        xbf_p = _Pool(tc, name="xbf", bufs=2)      # lp and vp slabs (bf16)
        xTbf_p = _Pool(tc, name="xTbf", bufs=1)    # cycled: lpT then vpT
        in_p = _Pool(tc, name="in8", bufs=1)
        w8_p = _Pool(tc, name="w8", bufs=2)

        def psum_tile(n):
            t = ps_p.tile([P, d], f32, tag="ps")
            return t[:, :n]

        vlT_sb = in_p.tile([P, 2 * dt_n, s], f8, tag="in8", name="vlT_sb")
        # all input loads up front on the sync ring, l k-tiles first
        # (projection order is l then v), first halves before second halves
        for h in range(2):
            for j in list(range(dt_n, 2 * dt_n)) + list(range(dt_n)):
                nc.sync.dma_start(
                    out=vlT_sb[:, j, h * (s // 2):(h + 1) * (s // 2)],
                    in_=vlT_d.ap()[j * P:(j + 1) * P,
                                   h * (s // 2):(h + 1) * (s // 2)])

        vpT8 = xT8_p.tile([P, dt_n, s], f8, tag='vpT8', name='vpT8')
        lpT8 = xT8_p.tile([P, dt_n, s], f8, tag='lpT8', name='lpT8')

        oW8_sb = ow8_p.tile([P, 2 * dt_n, dw], f8, tag="ow8", name="oW8_sb")
        oWb_sb = owb_p.tile([P, 2 * dt_n, dw], bf16, tag="owbf",
                            name="oWb_sb")

        def layernorm_relu_pair(pss, dsts, g_nm, be_nm, tagsfx,
                                dst_f32=False, use_pf=False):
            """LN+relu over [:, :d] of a pair of psum tiles.

            Row mean/var come from bn_stats+bn_aggr (one VectorE data pass
            per 384-wide group, exact fp32) -- no separate mean/Square
            passes.  The out passes read the psum directly.
            Scale-invariant: psum may be any uniform multiple of the true
            pre-LN activations (EPS is negligible vs var in both scalings).
            """
            n = len(pss)
            mv2 = sp.tile([P, 2, 2], f32, tag="mv" + tagsfx)
            pfs = []
            if use_pf and trivial:
                # proj path: one scalar Copy evacuates the psum within
                # ~0.6us; bn_stats runs on the bf16 copy (fast SBUF reads)
                for i, ps in enumerate(pss):
                    pf = pf_p.tile([P, d], bf16, tag="pf")
                    nc.scalar.activation(out=pf[:], in_=ps[:, :d],
                                         func=AF.Copy)
                    st12 = sp.tile([P, 2, 6], f32, tag="sb%d" % i + tagsfx)
                    nc.vector.bn_stats(out=st12[:, 0, :], in_=pf[:, 0:512])
                    nc.vector.bn_stats(out=st12[:, 1, :], in_=pf[:, 512:768])
                    nc.vector.bn_aggr(out=mv2[:, i, :], in_=st12[:])
                    pfs.append(pf)
            else:
                for i, ps in enumerate(pss):
                    st12 = sp.tile([P, 2, 6], f32, tag="st%d" % i + tagsfx)
                    nc.vector.bn_stats(out=st12[:, 0, :], in_=ps[:, 0:512])
                    nc.vector.bn_stats(out=st12[:, 1, :], in_=ps[:, 512:768])
                    nc.vector.bn_aggr(out=mv2[:, i, :], in_=st12[:])
            rstd = sp.tile([P, 2], f32, tag="rstd" + tagsfx)
            nc.scalar.activation(out=rstd[:, :n], in_=mv2[:, :n, 1],
                                 func=AF.Sqrt, bias=eps_sb[:])
            nc.vector.reciprocal(out=rstd[:, :n], in_=rstd[:, :n])
            mr = sp.tile([P, 2], f32, tag="mr" + tagsfx)
            nc.vector.tensor_tensor(out=mr[:, :n], in0=mv2[:, :n, 0],
                                    in1=rstd[:, :n], op=OP.mult)
            nmr = sp.tile([P, 2], f32, tag="nmr" + tagsfx)
            nc.vector.tensor_scalar(out=nmr[:, :n], in0=mr[:, :n],
                                    scalar1=-1.0, scalar2=None, op0=OP.mult)
            for i, (ps, dst) in enumerate(zip(pss, dsts)):
                if trivial:
                    if use_pf:
                        tmp = tmp_p.tile([P, d], bf16, tag="tmp")
                        nc.vector.tensor_scalar(out=tmp[:], in0=pfs[i][:],
                                                scalar1=rstd[:, i:i + 1],
                                                scalar2=mr[:, i:i + 1],
                                                op0=OP.mult,
                                                op1=OP.subtract)
                        nc.vector.tensor_scalar_max(out=dst, in0=tmp[:],
                                                    scalar1=0.0)
                        continue
                    # fused relu((y - m) * rstd) on ScalarE; VectorE keeps
                    # only the stats so both engines stay under the PE rate
                    nc.scalar.activation(out=dst, in_=ps[:, :d],
                                         func=AF.Relu,
                                         bias=nmr[:, i:i + 1],
                                         scale=rstd[:, i:i + 1])
                else:
                    nrm = sp.tile([P, d], f32, tag="nrm" + tagsfx, bufs=2)
                    nc.vector.tensor_scalar(out=nrm[:], in0=ps[:, :d],
                                            scalar1=rstd[:, i:i + 1],
                                            scalar2=mr[:, i:i + 1],
                                            op0=OP.mult, op1=OP.subtract)
                    nc.vector.tensor_mul(out=nrm[:], in0=nrm[:],
                                         in1=aff[g_nm][:])
                    nc.vector.tensor_add(out=nrm[:], in0=nrm[:],
                                         in1=aff[be_nm][:])
                    nc.vector.tensor_scalar_max(out=dst, in0=nrm[:],
                                                scalar1=0.0)

        # ---------- phase A/B: projections (fp8 DoubleRow) ----------
        def load_w(W_d):
            W_sb = w8_p.tile([P, dt_n, dw], f8, tag="w8", name="W_sb")
            for j in range(dt_n):
                nc.gpsimd.dma_start(out=W_sb[:, j, :],
                                    in_=W_d.ap()[j * P:(j + 1) * P, :])
            return W_sb

        def proj(base, W_sb, xpT8_dst, bias_nm, g_nm, be_nm,
                 defer_casts=False):
            xp_bf = xbf_p.tile([P, st_n, d], bf16, tag='xbf', name='xp_bf')
            xpT_bf = xTbf_p.tile([P, dt_n, s], bf16, tag='xTbf',
                                 name='xpT_bf')
            for st0 in range(0, st_n, 2):
                pss, dsts = [], []
                for st in (st0, st0 + 1):
                    ps = psum_tile(d)
                    for c0, cl in g_chunks:
                        for q in range(dt_n // 2):
                            nc.tensor.matmul(
                                ps[:, c0:c0 + cl],
                                vlT_sb[:, base + 2 * q:base + 2 * q + 2,
                                       st * P:(st + 1) * P],
                                W_sb[:, 2 * q:2 * q + 2, c0:c0 + cl],
                                start=(q == 0),
                                stop=(q == dt_n // 2 - 1 and trivial),
                                perf_mode=DR)
                        if not trivial:
                            nc.tensor.matmul(
                                ps[:, c0:c0 + cl], ones_sb[:1, :],
                                b_sb[bias_nm][:1, c0:c0 + cl],
                                start=False, stop=True)
                    pss.append(ps)
                    dsts.append(xp_bf[:, st, :])
                layernorm_relu_pair(pss, dsts, g_nm, be_nm, "p",
                                    use_pf=True)
                for st in (st0, st0 + 1):
                    ring = nc.scalar if st % 2 == 0 else nc.sync
                    ring.dma_start_transpose(
                        out=xpT_bf[:, :, st * P:(st + 1) * P],
                        in_=xp_bf[:, st, :])
            if not defer_casts:
                for j in range(dt_n):
                    nc.vector.tensor_copy(out=xpT8_dst[:, j, :],
                                          in_=xpT_bf[:, j, :])
            return xp_bf, xpT_bf

        def colsum_row(x_bf, row_dst):
            # column sums as a [1, d] row via a PE ones-matvec (keeps the
            # big reduces off VectorE)
            ps = psum_tile(d)
            for c0, cl in g_chunks:
                for q in range(st_n):
                    nc.tensor.matmul(ps[:1, c0:c0 + cl], onescol[:, :1],
                                     x_bf[:, q, c0:c0 + cl],
                                     start=(q == 0), stop=(q == st_n - 1))
            nc.scalar.activation(out=row_dst[:1, :], in_=ps[:1, :d],
                                 func=AF.Copy)

        def row_to_cols(row, col_specs):
            # PE-transpose a [1, d] row into [128, 1] columns: k=1 matmul of
            # the row slice against a single one, then scaled PSUM reads
            for j in range(dt_n):
                ps = psum_tile(16)
                nc.tensor.matmul(ps[:, :1], row[:1, j * P:(j + 1) * P],
                                 ones1b[:1, :1], start=True, stop=True)
                for dst, scl in col_specs:
                    nc.vector.tensor_scalar(out=dst[:, j, :], in0=ps[:, :1],
                                            scalar1=scl, scalar2=None,
                                            op0=OP.mult)

        Wl_sb = load_w(lW_d)
        lp_bf, _ = proj(dt_n, Wl_sb, lpT8, "lb", "lg", "lbeta")
        Wv_sb = load_w(vW_d)
        vp_bf, vpT_bf = proj(0, Wv_sb, vpT8, "vb", "vg", "vbeta",
                             defer_casts=True)
        # colsums after proj-v so its matmuls aren't queued behind them on
        # the in-order PE; the lp colsum also fills the vp-LN drain
        colsum_row(lp_bf, lcol_row)
        row_to_cols(lcol_row, [(lcolK, MS), (lcoln, -1.0 / s)])
        colsum_row(vp_bf, vcol_row)
        row_to_cols(vcol_row, [(vcolK, MS)])
        # oW loads on the gpsimd ring (sync carries half the transposes)
        for j in range(2 * dt_n):
            nc.gpsimd.dma_start(out=oW8_sb[:, j, :],
                                in_=oW8_d.ap()[j * P:(j + 1) * P, :])
            nc.gpsimd.dma_start(out=oWb_sb[:, j, :],
                                in_=oWb_d.ap()[j * P:(j + 1) * P, :])
        # input/weight slabs are dead now (xbf/xTbf stay: G and the
        # deferred vpT8 casts read them)
        w8_p.close()
        in_p.close()

        # ---------- phase C: G = lp^T vp (fp8 DR), transpose, folds --------
        g_p = _Pool(tc, name="gp", bufs=1)
        ot_p = _Pool(tc, name="outsb", bufs=2)
        G8 = g_p.tile([P, dt_n, d], f8, tag='G8', name='G8')
        G_bf = g_p.tile([P, dt_n, d], bf16, tag='G_bf', name='G_bf')
        GT_bf = g_p.tile([P, dt_n, d], bf16, tag='GT_bf', name='GT_bf')
        GT8 = g_p.tile([P, dt_n, d], f8, tag='GT8', name='GT8')
        Gt8 = g_p.tile([P, dt_n, dw], f8, tag='Gt8', name='Gt8')
        Gb8 = g_p.tile([P, dt_n, dw], f8, tag='Gb8', name='Gb8')
        Gb_bf = g_p.tile([P, dt_n, dw], bf16, tag='Gb_bf', name='Gb_bf')

        for j in range(dt_n):
            ps = psum_tile(d)
            for c0, cl in g_chunks:
                for q in range(st_n):
                    nc.tensor.matmul(
                        ps[:, c0:c0 + cl],
                        lp_bf[:, q, j * P:(j + 1) * P],
                        vp_bf[:, q, c0:c0 + cl],
                        start=(q == 0), stop=(q == st_n - 1))
            # G8 at half scale (G max ~452 vs fp8 max 448); G_bf exact
            nc.scalar.activation(out=G8[:, j, :], in_=ps[:, :d],
                                 func=AF.Copy, scale=0.5)
            nc.vector.tensor_copy(out=G_bf[:, j, :], in_=ps[:, :d])
            # deferred vpT8 cast rides the G phase's vector slack instead of
            # blocking the proj-v LN tail
            nc.vector.tensor_copy(out=vpT8[:, j, :], in_=vpT_bf[:, j, :])
        for j in range(dt_n):
            nc.scalar.dma_start_transpose(
                out=GT_bf[:, :, j * P:(j + 1) * P], in_=G_bf[:, j, :])
        for j in range(dt_n):
            nc.vector.tensor_scalar(out=GT8[:, j, :], in0=GT_bf[:, j, :],
                                    scalar1=0.5, scalar2=None, op0=OP.mult)

        # u = 256*vcol @ oW_t ; store uneg_row = -u/(2S) (rank-1 operand at
        # the G8 half-scale so one evacuation scale covers the Gt psum).
        # Placed after G so the vcol XBAR/cast latency hides under it.
        ps = psum_tile(d)
        for c0, cl in g_chunks:
            for k in range(dt_n):
                nc.tensor.matmul(ps[:1, c0:c0 + cl], vcolK[:, k, :],
                                 oWb_sb[:, k, c0:c0 + cl],
                                 start=(k == 0), stop=(k == dt_n - 1))
        nc.scalar.activation(out=uneg_row[:1, :d], in_=ps[:1, :d],
                             func=AF.Copy, scale=-1.0 / (2.0 * MS * s))

        # Gb = G^T @ oW_b  (stored fp8 at 2*K8*c, plus bf16 copy for CONST)
        for j in range(dt_n):
            ps = psum_tile(d)
            for c0, cl in g_chunks:
                for q in range(dt_n // 2):
                    nc.tensor.matmul(
                        ps[:, c0:c0 + cl],
                        G8[:, 2 * q:2 * q + 2, j * P:(j + 1) * P],
                        oW8_sb[:, dt_n + 2 * q:dt_n + 2 * q + 2, c0:c0 + cl],
                        start=(q == 0), stop=(q == dt_n // 2 - 1),
                        perf_mode=DR)
            nc.scalar.activation(out=Gb8[:, j, :d], in_=ps[:, :d],
                                 func=AF.Copy, scale=2.0 * K8 * cc)
            nc.vector.tensor_scalar(out=Gb_bf[:, j, :d], in0=ps[:, :d],
                                    scalar1=2.0 * K8 * cc, scalar2=None,
                                    op0=OP.mult)

        # Gt = G @ oW_t - (1/S) lcol x u   (rank-1 via k=1 bf16 matmul)
        for j in range(dt_n):
            ps = psum_tile(d)
            for c0, cl in g_chunks:
                for q in range(dt_n // 2):
                    nc.tensor.matmul(
                        ps[:, c0:c0 + cl],
                        GT8[:, 2 * q:2 * q + 2, j * P:(j + 1) * P],
                        oW8_sb[:, 2 * q:2 * q + 2, c0:c0 + cl],
                        start=(q == 0), stop=False,
                        perf_mode=DR)
                nc.tensor.matmul(
                    ps[:, c0:c0 + cl],
                    lcol_row[:1, j * P:(j + 1) * P],
                    uneg_row[:1, c0:c0 + cl],
                    start=False, stop=True)
            nc.scalar.activation(out=Gt8[:, j, :d], in_=ps[:, :d],
                                 func=AF.Copy, scale=2.0 * K8 * cc)

        # CONST row (256x scale): 256*vcol@oW_t + 256*lcol@oW_b
        #                         - (1/S) lcol@(2*K8*c*Gb) ; + 256*S*ob
        if not trivial:
            obS = sp.tile([1, dw], bf16, tag="obS")
            nc.vector.tensor_scalar(out=obS[:], in0=b_sb["ob"][:],
                                    scalar1=MS * float(s), scalar2=None,
                                    op0=OP.mult)
        ps = psum_tile(d)
        for c0, cl in g_chunks:
            for k in range(dt_n):
                nc.tensor.matmul(ps[:1, c0:c0 + cl], vcolK[:, k, :],
                                 oWb_sb[:, k, c0:c0 + cl],
                                 start=(k == 0), stop=False)
            for k in range(dt_n):
                nc.tensor.matmul(ps[:1, c0:c0 + cl], lcolK[:, k, :],
                                 oWb_sb[:, dt_n + k, c0:c0 + cl],
                                 start=False, stop=False)
            for k in range(dt_n):
                nc.tensor.matmul(ps[:1, c0:c0 + cl], lcoln[:, k, :],
                                 Gb_bf[:, k, c0:c0 + cl],
                                 start=False,
                                 stop=(trivial and k == dt_n - 1))
            if not trivial:
                nc.tensor.matmul(ps[:1, c0:c0 + cl], ones_sb[:1, :1],
                                 obS[:1, c0:c0 + cl], start=False, stop=True)
        nc.scalar.activation(out=chi_r[:1, :d], in_=ps[:1, :d],
                             func=AF.Copy)
        nc.vector.tensor_tensor(out=clo_r[:1, :d], in0=ps[:1, :d],
                                in1=chi_r[:1, :d], op=OP.subtract)
        nc.gpsimd.dma_start(out=chilo2[0:1, :d], in_=chi_r[:1, :d])
        nc.gpsimd.dma_start(out=chilo2[1:2, :d], in_=clo_r[:1, :d])

        # ---------- phase D: final  psum = vp@Gt + lp@Gb + 1 x (hi+lo) ------
        for st0 in range(0, st_n, 2):
            pss, ots = [], []
            for st in (st0, st0 + 1):
                ps = psum_tile(d)
                for c0, cl in g_chunks:
                    for q in range(dt_n // 2):
                        nc.tensor.matmul(
                            ps[:, c0:c0 + cl],
                            vpT8[:, 2 * q:2 * q + 2, st * P:(st + 1) * P],
                            Gt8[:, 2 * q:2 * q + 2, c0:c0 + cl],
                            start=(q == 0), stop=False, perf_mode=DR)
                    for q in range(dt_n // 2):
                        nc.tensor.matmul(
                            ps[:, c0:c0 + cl],
                            lpT8[:, 2 * q:2 * q + 2, st * P:(st + 1) * P],
                            Gb8[:, 2 * q:2 * q + 2, c0:c0 + cl],
                            start=False, stop=False, perf_mode=DR)
                    nc.tensor.matmul(
                        ps[:, c0:c0 + cl],
                        ones2[:2, st * P:(st + 1) * P],
                        chilo2[:2, c0:c0 + cl],
                        start=False, stop=True)
                pss.append(ps)
                ots.append(ot_p.tile([P, d], bf16, tag="ot", bufs=3))
            layernorm_relu_pair(pss, [o[:] for o in ots],
                                "og", "obeta", "o", dst_f32=True)
            for i, st in enumerate((st0, st0 + 1)):
                nc.sync.dma_start(
                    out=out_d.ap()[st * P:(st + 1) * P, :], in_=ots[i][:])

        ot_p.close()
        g_p.close()
        xTbf_p.close()
        xbf_p.close()
        ps_p.close()
        owb_p.close()
        ow8_p.close()
        xT8_p.close()
        pf_p.close()
        tmp_p.close()
        sp.close()
        pp.close()
        # (ps_p is on the separate PSUM stack; SBUF closes above are LIFO)

    nc.compile()
    return nc


def _get_program(c_scale: float, trivial: bool, s: int = S, d: int = D):
    key = (round(float(c_scale), 12), trivial, s, d)
    if key not in _BUILD_CACHE:
        _BUILD_CACHE[key] = _build(c_scale, trivial, s, d)
    return _BUILD_CACHE[key]


def _w_ext(W, dtype):
    """[K, N] weights -> [K, N + DPAD] with col N = MS * row-mean, pad 0."""
    W = np.asarray(W, np.float32)
    k = W.shape[0]
    ext = np.zeros((k, W.shape[1] + DPAD), np.float32)
    ext[:, :W.shape[1]] = W
    ext[:, W.shape[1]] = MS * W.mean(axis=1)
    return np.ascontiguousarray(ext.astype(dtype))


def _prep_in_maps(vision, language, vW, lW, oW, trivial, extras):
    n_b = vision.shape[0]
    vW8 = _w_ext(vW, F8E4)
    lW8 = _w_ext(lW, F8E4)
    oW_ext = _w_ext(oW, np.float32)
    oW_ext[:, D] = 0.0            # final LN mean is computed on device
    oW8 = np.ascontiguousarray(oW_ext.astype(F8E4))
    oWbf = np.ascontiguousarray(oW_ext.astype(BF16))
    in_maps = []
    for b in range(n_b):
        vlT = np.concatenate([vision[b].T, language[b].T], 0)
        m = {
            "vlT8": np.ascontiguousarray(vlT.astype(F8E4)),
            "vW8": vW8, "lW8": lW8, "oW8": oW8, "oWbf": oWbf,
        }
        if not trivial:
            m.update(extras)
        in_maps.append(m)
    return in_maps


def _program_and_inmaps(inputs):
    """(compiled program, per-core input maps) for the given full inputs."""
    vision = np.asarray(inputs["vision_features"], np.float32)
    language = np.asarray(inputs["language_features"], np.float32)
    c_scale = float(np.asarray(inputs["claw"], np.float32).mean()) / TEMPERATURE
    nc = _get_program(c_scale, True)
    in_maps = _prep_in_maps(vision, language, inputs["vW"], inputs["lW"],
                            inputs["oW"], True, {})
    return nc, in_maps


def kernel(vision_features, language_features, vW, vb, vg, vbeta,
           lW, lb, lg, lbeta, claw, oW, ob, og, obeta):
    from concourse import bass_utils

    vision = np.asarray(vision_features, np.float32)
    language = np.asarray(language_features, np.float32)
    c_scale = float(np.asarray(claw, np.float32).mean()) / TEMPERATURE
    # linearized softmax: valid when the logit scale is small (|y| << 1);
    # |sim| <= 1.5*D is a conservative row-norm bound
    assert abs(c_scale) * 1.5 * D < 0.8, "logit scale too large to linearize"

    trivial = (
        np.all(np.asarray(vb) == 0) and np.all(np.asarray(lb) == 0)
        and np.all(np.asarray(ob) == 0)
        and np.all(np.asarray(vg) == 1) and np.all(np.asarray(vbeta) == 0)
        and np.all(np.asarray(lg) == 1) and np.all(np.asarray(lbeta) == 0)
        and np.all(np.asarray(og) == 1) and np.all(np.asarray(obeta) == 0)
    )

    def bias_ext(bv):
        bv = np.asarray(bv, np.float32).reshape(D)
        ext = np.zeros(D + DPAD, np.float32)
        ext[:D] = bv
        ext[D] = MS * bv.mean()
        return ext.reshape(1, D + DPAD).astype(BF16)

    extras = {}
    if not trivial:
        extras = {
            "vb": bias_ext(vb), "lb": bias_ext(lb), "ob": bias_ext(ob),
            "vg": np.asarray(vg, np.float32).reshape(1, D),
            "vbeta": np.asarray(vbeta, np.float32).reshape(1, D),
            "lg": np.asarray(lg, np.float32).reshape(1, D),
            "lbeta": np.asarray(lbeta, np.float32).reshape(1, D),
            "og": np.asarray(og, np.float32).reshape(1, D),
            "obeta": np.asarray(obeta, np.float32).reshape(1, D),
        }

    nc = _get_program(c_scale, trivial)
    in_maps = _prep_in_maps(vision, language, vW, lW, oW, trivial, extras)
    res = bass_utils.run_bass_kernel_spmd(nc, in_maps,
                                          core_ids=list(range(B)))
    return np.stack([np.asarray(res.results[b]["out"], np.float32)
                     for b in range(B)], axis=0)


# revision 66
# speedup vs baseline: 1.1565x; 1.0038x over previous
"""Trainium2 Bass kernel for nn_ClawMatrix (cross-modal claw-matrix alignment).

reference per batch element b:
    vp = relu(LN(v @ vW))                [S, D]
    lp = relu(LN(l @ lW))                [S, D]
    sim = vp @ lp.T ; x = c * sim        c = mean(claw)/0.07  (~ -5.8e-4)
    A = softmax(x, axis=-1)
    out = relu(LN([A @ vp, A.T @ lp] @ oW))

Key algebraic identity: the logits x are tiny (|x| < 0.11), and softmax is
shift-invariant so only the per-row deviation y = x - rowmean(x) matters
(|y| <~ 0.06).  Linearizing exp(y) ~= 1 + y makes the softmax row-sum
EXACTLY S, so

    A[s,t] = (1 + c*sim[s,t] - m_s) / S,   m_s = (c/S) * vp_s . lcol

(lcol = column sums of lp).  Both S x S bmms and sim itself then collapse
into D x D matmuls via G = lp^T vp:

    out_pre * S = CONST(row) + c*vp@(G@oW_t - (1/S) lcol x u) + c*lp@(G^T@oW_b)
    CONST = u + lcol@oW_b - (c/S) lcol@(G^T@oW_b),   u = vcol@oW_t

The final LN is scale-invariant, so the 1/S and 256x fp8 scales are never
divided out.  Numerically validated vs the fp64 reference: linearization
error 3e-6; full pipeline ~5e-3 on hardware (budget 2e-2).

Device strategy (one batch element per core, 8 cores, no collectives;
baseline 372.8us -> ~240us):
  - projections, Gt/Gb folds and the final [S,2D]@[2D,D] matmul run
    fp8e4m3 with perf_mode=DoubleRow (k-pair 3D APs, stride%16==0); G runs
    bf16 straight from the projection slabs (no extra fp8 copies)
  - LN: a single scalar Copy evacuates each psum tile to bf16 within
    ~0.6us (PSUM slots are the PE's runway -- keeping the PE continuously
    busy matters doubly on TRN2 because the tensor clock p-state ramps
    0.65 -> 1.2 -> 2.4 GHz and resets on every idle gap); mean/var via
    vector bn_stats/bn_aggr on the bf16 copy (projections) or straight
    from psum (final, exact fp32); relu fused into ScalarE activations
    for the final phase and VectorE mult-sub+max passes for the
    projections
  - vp/lp transposed via XBAR in bf16 (2-byte elements only) then cast to
    fp8; transposes alternate between the two HWDGE rings (sync/scalar)
    because a ring's semaphore wait occupies its host engine's sequencer
  - column sums via PE ones-matvecs; rows -> columns via k=1 matmuls
    against a single one (XBAR cannot transpose 1-row tiles)
  - CONST row stored as bf16 hi+lo on two partitions (stacked by a one-off
    DMA) so one k=2 matmul applies it at ~16-bit precision
  - output written bf16 (host casts to f32); engine work balanced so
    VectorE/ScalarE/both DMA rings each stay under the PE pair cadence
"""

import os
import sys
import numpy as np

for _p in ("/opt/trn_rl_repo", "/root/.axon_site/_ro/trn_rl_repo"):
    if os.path.isdir(_p) and _p not in sys.path:
        sys.path.insert(0, _p)

import ml_dtypes  # noqa: E402

BF16 = ml_dtypes.bfloat16
F8E4 = ml_dtypes.float8_e4m3

P = 128           # partitions
B = 8             # batch / cores
S = 2048          # sequence
D = 768           # feature dim
DPAD = 16         # weight pad: [mean col | zeros]; keeps DR stride %16==0
DW = D + DPAD     # 784
MS = 256.0        # mean-column scale (fp8 subnormal guard)
K8 = 256.0        # fp8 storage scale for Gt/Gb (and CONST, final psum)
EPS = 1e-5
TEMPERATURE = 0.07

_BUILD_CACHE = {}


class _Pool:
    """Manually managed tile-pool lifetime (enter now, exit at any point)."""

    def __init__(self, tc, **kw):
        self._cm = tc.tile_pool(**kw)
        self.pool = self._cm.__enter__()
        self._open = True

    def tile(self, *a, **kw):
        if "name" not in kw:
            kw["name"] = kw.get("tag") or f"t{id(self) % 9973}"
        return self.pool.tile(*a, **kw)

    def close(self):
        if self._open:
            self._cm.__exit__(None, None, None)
            self._open = False


def _build(c_scale: float, trivial: bool, s: int = S, d: int = D):
    """Builds the single-core Bass program. Returns the compiled Bacc module."""
    import concourse.bass as bass
    import concourse.tile as tile
    from concourse import bacc, mybir

    f32 = mybir.dt.float32
    bf16 = mybir.dt.bfloat16
    f8 = mybir.dt.float8e4
    DR = mybir.MatmulPerfMode.DoubleRow
    AF = mybir.ActivationFunctionType
    AX = mybir.AxisListType
    OP = mybir.AluOpType

    st_n = s // P          # 16 row tiles over S
    dt_n = d // P          # 6 tiles over D
    dw = d + DPAD
    d_chunks = [(0, 512), (512, dw - 512)]     # psum-bank chunks over DW
    g_chunks = [(0, 512), (512, d - 512)]
    cc = float(c_scale)

    nc = bacc.Bacc(
        "TRN2",
        target_bir_lowering=False,
        debug=False,
        enable_asserts=False,
        num_devices=B,
    )

    vlT_d = nc.dram_tensor("vlT8", [2 * d, s], f8, kind="ExternalInput")
    vW_d = nc.dram_tensor("vW8", [d, dw], f8, kind="ExternalInput")
    lW_d = nc.dram_tensor("lW8", [d, dw], f8, kind="ExternalInput")
    oW8_d = nc.dram_tensor("oW8", [2 * d, dw], f8, kind="ExternalInput")
    oWb_d = nc.dram_tensor("oWbf", [2 * d, dw], bf16, kind="ExternalInput")
    if not trivial:
        vb_d = nc.dram_tensor("vb", [1, dw], bf16, kind="ExternalInput")
        lb_d = nc.dram_tensor("lb", [1, dw], bf16, kind="ExternalInput")
        ob_d = nc.dram_tensor("ob", [1, dw], bf16, kind="ExternalInput")
        vg_d = nc.dram_tensor("vg", [1, d], f32, kind="ExternalInput")
        vbe_d = nc.dram_tensor("vbeta", [1, d], f32, kind="ExternalInput")
        lg_d = nc.dram_tensor("lg", [1, d], f32, kind="ExternalInput")
        lbe_d = nc.dram_tensor("lbeta", [1, d], f32, kind="ExternalInput")
        og_d = nc.dram_tensor("og", [1, d], f32, kind="ExternalInput")
        obe_d = nc.dram_tensor("obeta", [1, d], f32, kind="ExternalInput")
    out_d = nc.dram_tensor("out", [s, d], bf16, kind="ExternalOutput")

    with tile.TileContext(nc) as tc:
        pp = _Pool(tc, name="persist", bufs=1)
        sp = _Pool(tc, name="small", bufs=4)
        tmp_p = _Pool(tc, name="tmp", bufs=3)
        pf_p = _Pool(tc, name="pf", bufs=4)

        eps_sb = pp.tile([P, 1], f32, tag='eps')
        nc.vector.memset(eps_sb[:], EPS)
        ones2 = pp.tile([2, s], bf16, tag='ones2')
        nc.vector.memset(ones2[:], 1.0)
        onescol = pp.tile([P, 1], bf16, tag='onescol')
        nc.vector.memset(onescol[:], 1.0)

        if not trivial:
            ones_sb = pp.tile([1, P], bf16, tag="ones_sb")
            nc.vector.memset(ones_sb[:], 1.0)
            b_sb = {}
            aff = {}
            for nm, dd in (("vb", vb_d), ("lb", lb_d), ("ob", ob_d)):
                t = pp.tile([1, dw], bf16, tag=nm)
                nc.sync.dma_start(out=t[:], in_=dd.ap())
                b_sb[nm] = t
            for nm, dd in (("vg", vg_d), ("vbeta", vbe_d), ("lg", lg_d),
                           ("lbeta", lbe_d), ("og", og_d), ("obeta", obe_d)):
                t = pp.tile([P, d], f32, tag=nm)
                src = bass.AP(tensor=dd.ap().tensor, offset=0,
                              ap=[[0, P], [1, d]])
                nc.sync.dma_start(out=t[:], in_=src)
                aff[nm] = t

        # column/row statistics tiles ([P, 6, 1] column layout; the rows are
        # PE-transposed into columns via k=1 matmuls against ones1b)
        vcolK = pp.tile([P, dt_n, 1], bf16, tag='vcolK')   # 256*vcol
        lcolK = pp.tile([P, dt_n, 1], bf16, tag='lcolK')   # 256*lcol
        lcoln = pp.tile([P, dt_n, 1], bf16, tag='lcoln')   # -lcol/S
        vcol_row = pp.tile([1, d], bf16, tag='vcol_row')
        lcol_row = pp.tile([1, d], bf16, tag='lcol_row')
        ones1b = pp.tile([1, 1], bf16, tag='ones1b')
        nc.vector.memset(ones1b[:], 1.0)
        uneg_row = pp.tile([1, dw], bf16, tag='uneg_row')  # -u/(2S)
        chi_r = pp.tile([1, dw], bf16, tag='chi_r')        # CONST hi row
        clo_r = pp.tile([1, dw], bf16, tag='clo_r')        # CONST lo row
        chilo2 = pp.tile([2, dw], bf16, tag='chilo2')      # stacked hi/lo

        # big SBUF slabs.  Pool closes must be LIFO per memory space, so the
        # transient projection-era pools (in8/w8/xbf/xTbf) are created LAST
        # and closed mid-program in reverse order, freeing their region for
        # the g_p pool pushed after them.
        xT8_p = _Pool(tc, name="xT8", bufs=1)
        ow8_p = _Pool(tc, name="ow8", bufs=1)
        owb_p = _Pool(tc, name="owbf", bufs=1)
        ps_p = _Pool(tc, name="psuni", bufs=4, space=bass.MemorySpace.PSUM)
        xbf_p = _Pool(tc, name="xbf", bufs=2)      # lp and vp slabs (bf16)
        xTbf_p = _Pool(tc, name="xTbf", bufs=1)    # cycled: lpT then vpT
        in_p = _Pool(tc, name="in8", bufs=1)
        w8_p = _Pool(tc, name="w8", bufs=2)

        def psum_tile(n):
            t = ps_p.tile([P, d], f32, tag="ps")
            return t[:, :n]

        vlT_sb = in_p.tile([P, 2 * dt_n, s], f8, tag="in8", name="vlT_sb")
        # all input loads up front on the sync ring, l k-tiles first
        # (projection order is l then v), first halves before second halves
        for h in range(2):
            for j in list(range(dt_n, 2 * dt_n)) + list(range(dt_n)):
                nc.sync.dma_start(
                    out=vlT_sb[:, j, h * (s // 2):(h + 1) * (s // 2)],
                    in_=vlT_d.ap()[j * P:(j + 1) * P,
                                   h * (s // 2):(h + 1) * (s // 2)])

        vpT8 = xT8_p.tile([P, dt_n, s], f8, tag='vpT8', name='vpT8')
        lpT8 = xT8_p.tile([P, dt_n, s], f8, tag='lpT8', name='lpT8')

        oW8_sb = ow8_p.tile([P, 2 * dt_n, dw], f8, tag="ow8", name="oW8_sb")
        oWb_sb = owb_p.tile([P, 2 * dt_n, dw], bf16, tag="owbf",
                            name="oWb_sb")

        def layernorm_relu_pair(pss, dsts, g_nm, be_nm, tagsfx,
                                dst_f32=False, use_pf=False):
            """LN+relu over [:, :d] of a pair of psum tiles.

            Row mean/var come from bn_stats+bn_aggr (one VectorE data pass
            per 384-wide group, exact fp32) -- no separate mean/Square
            passes.  The out passes read the psum directly.
            Scale-invariant: psum may be any uniform multiple of the true
            pre-LN activations (EPS is negligible vs var in both scalings).
            """
            n = len(pss)
            mv2 = sp.tile([P, 2, 2], f32, tag="mv" + tagsfx)
            pfs = []
            if use_pf and trivial:
                # proj path: one scalar Copy evacuates the psum within
                # ~0.6us; bn_stats runs on the bf16 copy (fast SBUF reads)
                for i, ps in enumerate(pss):
                    pf = pf_p.tile([P, d], bf16, tag="pf")
                    nc.scalar.activation(out=pf[:], in_=ps[:, :d],
                                         func=AF.Copy)
                    st12 = sp.tile([P, 2, 6], f32, tag="sb%d" % i + tagsfx)
                    nc.vector.bn_stats(out=st12[:, 0, :], in_=pf[:, 0:512])
                    nc.vector.bn_stats(out=st12[:, 1, :], in_=pf[:, 512:768])
                    nc.vector.bn_aggr(out=mv2[:, i, :], in_=st12[:])
                    pfs.append(pf)
            else:
                for i, ps in enumerate(pss):
                    st12 = sp.tile([P, 2, 6], f32, tag="st%d" % i + tagsfx)
                    nc.vector.bn_stats(out=st12[:, 0, :], in_=ps[:, 0:512])
                    nc.vector.bn_stats(out=st12[:, 1, :], in_=ps[:, 512:768])
                    nc.vector.bn_aggr(out=mv2[:, i, :], in_=st12[:])
            rstd = sp.tile([P, 2], f32, tag="rstd" + tagsfx)
            nc.scalar.activation(out=rstd[:, :n], in_=mv2[:, :n, 1],
                                 func=AF.Sqrt, bias=eps_sb[:])
            nc.vector.reciprocal(out=rstd[:, :n], in_=rstd[:, :n])
            mr = sp.tile([P, 2], f32, tag="mr" + tagsfx)
            nc.vector.tensor_tensor(out=mr[:, :n], in0=mv2[:, :n, 0],
                                    in1=rstd[:, :n], op=OP.mult)
            nmr = sp.tile([P, 2], f32, tag="nmr" + tagsfx)
            nc.vector.tensor_scalar(out=nmr[:, :n], in0=mr[:, :n],
                                    scalar1=-1.0, scalar2=None, op0=OP.mult)
            for i, (ps, dst) in enumerate(zip(pss, dsts)):
                if trivial:
                    if use_pf:
                        tmp = tmp_p.tile([P, d], bf16, tag="tmp")
                        nc.vector.tensor_scalar(out=tmp[:], in0=pfs[i][:],
                                                scalar1=rstd[:, i:i + 1],
                                                scalar2=mr[:, i:i + 1],
                                                op0=OP.mult,
                                                op1=OP.subtract)
                        nc.vector.tensor_scalar_max(out=dst, in0=tmp[:],
                                                    scalar1=0.0)
                        continue
                    # fused relu((y - m) * rstd) on ScalarE; VectorE keeps
                    # only the stats so both engines stay under the PE rate
                    nc.scalar.activation(out=dst, in_=ps[:, :d],
                                         func=AF.Relu,
                                         bias=nmr[:, i:i + 1],
                                         scale=rstd[:, i:i + 1])
                else:
                    nrm = sp.tile([P, d], f32, tag="nrm" + tagsfx, bufs=2)
                    nc.vector.tensor_scalar(out=nrm[:], in0=ps[:, :d],
                                            scalar1=rstd[:, i:i + 1],
                                            scalar2=mr[:, i:i + 1],
                                            op0=OP.mult, op1=OP.subtract)
                    nc.vector.tensor_mul(out=nrm[:], in0=nrm[:],
                                         in1=aff[g_nm][:])
                    nc.vector.tensor_add(out=nrm[:], in0=nrm[:],
                                         in1=aff[be_nm][:])
                    nc.vector.tensor_scalar_max(out=dst, in0=nrm[:],
                                                scalar1=0.0)

        # ---------- phase A/B: projections (fp8 DoubleRow) ----------
        def load_w(W_d):
            W_sb = w8_p.tile([P, dt_n, dw], f8, tag="w8", name="W_sb")
            for j in range(dt_n):
                nc.gpsimd.dma_start(out=W_sb[:, j, :],
                                    in_=W_d.ap()[j * P:(j + 1) * P, :])
            return W_sb

        def proj(base, W_sb, xpT8_dst, bias_nm, g_nm, be_nm,
                 defer_casts=False):
            xp_bf = xbf_p.tile([P, st_n, d], bf16, tag='xbf', name='xp_bf')
            xpT_bf = xTbf_p.tile([P, dt_n, s], bf16, tag='xTbf',
                                 name='xpT_bf')
            for st0 in range(0, st_n, 2):
                pss, dsts = [], []
                for st in (st0, st0 + 1):
                    ps = psum_tile(d)
                    for c0, cl in g_chunks:
                        for q in range(dt_n // 2):
                            nc.tensor.matmul(
                                ps[:, c0:c0 + cl],
                                vlT_sb[:, base + 2 * q:base + 2 * q + 2,
                                       st * P:(st + 1) * P],
                                W_sb[:, 2 * q:2 * q + 2, c0:c0 + cl],
                                start=(q == 0),
                                stop=(q == dt_n // 2 - 1 and trivial),
                                perf_mode=DR)
                        if not trivial:
                            nc.tensor.matmul(
                                ps[:, c0:c0 + cl], ones_sb[:1, :],
                                b_sb[bias_nm][:1, c0:c0 + cl],
                                start=False, stop=True)
                    pss.append(ps)
                    dsts.append(xp_bf[:, st, :])
                layernorm_relu_pair(pss, dsts, g_nm, be_nm, "p",
                                    use_pf=True)
                for st in (st0, st0 + 1):
                    ring = nc.scalar if st % 2 == 0 else nc.sync
                    ring.dma_start_transpose(
                        out=xpT_bf[:, :, st * P:(st + 1) * P],
                        in_=xp_bf[:, st, :])
            if not defer_casts:
                for j in range(dt_n):
                    if j % 2 == 0:
                        nc.vector.tensor_copy(out=xpT8_dst[:, j, :],
                                              in_=xpT_bf[:, j, :])
                    else:
                        nc.scalar.activation(out=xpT8_dst[:, j, :],
                                             in_=xpT_bf[:, j, :],
                                             func=AF.Copy)
            return xp_bf, xpT_bf

        def colsum_row(x_bf, row_dst):
            # column sums as a [1, d] row via a PE ones-matvec (keeps the
            # big reduces off VectorE)
            ps = psum_tile(d)
            for c0, cl in g_chunks:
                for q in range(st_n):
                    nc.tensor.matmul(ps[:1, c0:c0 + cl], onescol[:, :1],
                                     x_bf[:, q, c0:c0 + cl],
                                     start=(q == 0), stop=(q == st_n - 1))
            nc.scalar.activation(out=row_dst[:1, :], in_=ps[:1, :d],
                                 func=AF.Copy)

        def row_to_cols(row, col_specs):
            # PE-transpose a [1, d] row into [128, 1] columns: k=1 matmul of
            # the row slice against a single one, then scaled PSUM reads
            for j in range(dt_n):
                ps = psum_tile(16)
                nc.tensor.matmul(ps[:, :1], row[:1, j * P:(j + 1) * P],
                                 ones1b[:1, :1], start=True, stop=True)
                for dst, scl in col_specs:
                    nc.vector.tensor_scalar(out=dst[:, j, :], in0=ps[:, :1],
                                            scalar1=scl, scalar2=None,
                                            op0=OP.mult)

        Wl_sb = load_w(lW_d)
        lp_bf, _ = proj(dt_n, Wl_sb, lpT8, "lb", "lg", "lbeta")
        Wv_sb = load_w(vW_d)
        vp_bf, vpT_bf = proj(0, Wv_sb, vpT8, "vb", "vg", "vbeta",
                             defer_casts=True)
        # colsums after proj-v so its matmuls aren't queued behind them on
        # the in-order PE; the lp colsum also fills the vp-LN drain
        colsum_row(lp_bf, lcol_row)
        row_to_cols(lcol_row, [(lcolK, MS), (lcoln, -1.0 / s)])
        colsum_row(vp_bf, vcol_row)
        row_to_cols(vcol_row, [(vcolK, MS)])
        # oW loads on the gpsimd ring (sync carries half the transposes)
        for j in range(2 * dt_n):
            nc.gpsimd.dma_start(out=oW8_sb[:, j, :],
                                in_=oW8_d.ap()[j * P:(j + 1) * P, :])
            nc.gpsimd.dma_start(out=oWb_sb[:, j, :],
                                in_=oWb_d.ap()[j * P:(j + 1) * P, :])
        # input/weight slabs are dead now (xbf/xTbf stay: G and the
        # deferred vpT8 casts read them)
        w8_p.close()
        in_p.close()

        # ---------- phase C: G = lp^T vp (fp8 DR), transpose, folds --------
        g_p = _Pool(tc, name="gp", bufs=1)
        ot_p = _Pool(tc, name="outsb", bufs=2)
        G8 = g_p.tile([P, dt_n, d], f8, tag='G8', name='G8')
        G_bf = g_p.tile([P, dt_n, d], bf16, tag='G_bf', name='G_bf')
        GT_bf = g_p.tile([P, dt_n, d], bf16, tag='GT_bf', name='GT_bf')
        GT8 = g_p.tile([P, dt_n, d], f8, tag='GT8', name='GT8')
        Gt8 = g_p.tile([P, dt_n, dw], f8, tag='Gt8', name='Gt8')
        Gb8 = g_p.tile([P, dt_n, dw], f8, tag='Gb8', name='Gb8')
        Gb_bf = g_p.tile([P, dt_n, dw], bf16, tag='Gb_bf', name='Gb_bf')

        for j in range(dt_n):
            ps = psum_tile(d)
            for c0, cl in g_chunks:
                for q in range(st_n):
                    nc.tensor.matmul(
                        ps[:, c0:c0 + cl],
                        lp_bf[:, q, j * P:(j + 1) * P],
                        vp_bf[:, q, c0:c0 + cl],
                        start=(q == 0), stop=(q == st_n - 1))
            # G8 at half scale (G max ~452 vs fp8 max 448); G_bf exact
            nc.scalar.activation(out=G8[:, j, :], in_=ps[:, :d],
                                 func=AF.Copy, scale=0.5)
            nc.vector.tensor_copy(out=G_bf[:, j, :], in_=ps[:, :d])
            # deferred vpT8 cast rides the G phase's vector slack instead of
            # blocking the proj-v LN tail
            nc.vector.tensor_copy(out=vpT8[:, j, :], in_=vpT_bf[:, j, :])
        for j in range(dt_n):
            nc.scalar.dma_start_transpose(
                out=GT_bf[:, :, j * P:(j + 1) * P], in_=G_bf[:, j, :])
        for j in range(dt_n):
            nc.vector.tensor_scalar(out=GT8[:, j, :], in0=GT_bf[:, j, :],
                                    scalar1=0.5, scalar2=None, op0=OP.mult)

        # u = 256*vcol @ oW_t ; store uneg_row = -u/(2S) (rank-1 operand at
        # the G8 half-scale so one evacuation scale covers the Gt psum).
        # Placed after G so the vcol XBAR/cast latency hides under it.
        ps = psum_tile(d)
        for c0, cl in g_chunks:
            for k in range(dt_n):
                nc.tensor.matmul(ps[:1, c0:c0 + cl], vcolK[:, k, :],
                                 oWb_sb[:, k, c0:c0 + cl],
                                 start=(k == 0), stop=(k == dt_n - 1))
        nc.scalar.activation(out=uneg_row[:1, :d], in_=ps[:1, :d],
                             func=AF.Copy, scale=-1.0 / (2.0 * MS * s))

        # Gb = G^T @ oW_b  (stored fp8 at 2*K8*c, plus bf16 copy for CONST)
        for j in range(dt_n):
            ps = psum_tile(d)
            for c0, cl in g_chunks:
                for q in range(dt_n // 2):
                    nc.tensor.matmul(
                        ps[:, c0:c0 + cl],
                        G8[:, 2 * q:2 * q + 2, j * P:(j + 1) * P],
                        oW8_sb[:, dt_n + 2 * q:dt_n + 2 * q + 2, c0:c0 + cl],
                        start=(q == 0), stop=(q == dt_n // 2 - 1),
                        perf_mode=DR)
            nc.scalar.activation(out=Gb8[:, j, :d], in_=ps[:, :d],
                                 func=AF.Copy, scale=2.0 * K8 * cc)
            nc.vector.tensor_scalar(out=Gb_bf[:, j, :d], in0=ps[:, :d],
                                    scalar1=2.0 * K8 * cc, scalar2=None,
                                    op0=OP.mult)

        # Gt = G @ oW_t - (1/S) lcol x u   (rank-1 via k=1 bf16 matmul)
        for j in range(dt_n):
            ps = psum_tile(d)
            for c0, cl in g_chunks:
                for q in range(dt_n // 2):
                    nc.tensor.matmul(
                        ps[:, c0:c0 + cl],
                        GT8[:, 2 * q:2 * q + 2, j * P:(j + 1) * P],
                        oW8_sb[:, 2 * q:2 * q + 2, c0:c0 + cl],
                        start=(q == 0), stop=False,
                        perf_mode=DR)
                nc.tensor.matmul(
                    ps[:, c0:c0 + cl],
                    lcol_row[:1, j * P:(j + 1) * P],
                    uneg_row[:1, c0:c0 + cl],
                    start=False, stop=True)
            nc.scalar.activation(out=Gt8[:, j, :d], in_=ps[:, :d],
                                 func=AF.Copy, scale=2.0 * K8 * cc)

        # CONST row (256x scale): 256*vcol@oW_t + 256*lcol@oW_b
        #                         - (1/S) lcol@(2*K8*c*Gb) ; + 256*S*ob
        if not trivial:
            obS = sp.tile([1, dw], bf16, tag="obS")
            nc.vector.tensor_scalar(out=obS[:], in0=b_sb["ob"][:],
                                    scalar1=MS * float(s), scalar2=None,
                                    op0=OP.mult)
        ps = psum_tile(d)
        for c0, cl in g_chunks:
            for k in range(dt_n):
                nc.tensor.matmul(ps[:1, c0:c0 + cl], vcolK[:, k, :],
                                 oWb_sb[:, k, c0:c0 + cl],
                                 start=(k == 0), stop=False)
            for k in range(dt_n):
                nc.tensor.matmul(ps[:1, c0:c0 + cl], lcolK[:, k, :],
                                 oWb_sb[:, dt_n + k, c0:c0 + cl],
                                 start=False, stop=False)
            for k in range(dt_n):
                nc.tensor.matmul(ps[:1, c0:c0 + cl], lcoln[:, k, :],
                                 Gb_bf[:, k, c0:c0 + cl],
                                 start=False,
                                 stop=(trivial and k == dt_n - 1))
            if not trivial:
                nc.tensor.matmul(ps[:1, c0:c0 + cl], ones_sb[:1, :1],
                                 obS[:1, c0:c0 + cl], start=False, stop=True)
        nc.scalar.activation(out=chi_r[:1, :d], in_=ps[:1, :d],
                             func=AF.Copy)
        nc.vector.tensor_tensor(out=clo_r[:1, :d], in0=ps[:1, :d],
                                in1=chi_r[:1, :d], op=OP.subtract)
        nc.gpsimd.dma_start(out=chilo2[0:1, :d], in_=chi_r[:1, :d])
        nc.gpsimd.dma_start(out=chilo2[1:2, :d], in_=clo_r[:1, :d])

        # ---------- phase D: final  psum = vp@Gt + lp@Gb + 1 x (hi+lo) ------
        for st0 in range(0, st_n, 2):
            pss, ots = [], []
            for st in (st0, st0 + 1):
                ps = psum_tile(d)
                for c0, cl in g_chunks:
                    for q in range(dt_n // 2):
                        nc.tensor.matmul(
                            ps[:, c0:c0 + cl],
                            vpT8[:, 2 * q:2 * q + 2, st * P:(st + 1) * P],
                            Gt8[:, 2 * q:2 * q + 2, c0:c0 + cl],
                            start=(q == 0), stop=False, perf_mode=DR)
                    for q in range(dt_n // 2):
                        nc.tensor.matmul(
                            ps[:, c0:c0 + cl],
                            lpT8[:, 2 * q:2 * q + 2, st * P:(st + 1) * P],
                            Gb8[:, 2 * q:2 * q + 2, c0:c0 + cl],
                            start=False, stop=False, perf_mode=DR)
                    nc.tensor.matmul(
                        ps[:, c0:c0 + cl],
                        ones2[:2, st * P:(st + 1) * P],
                        chilo2[:2, c0:c0 + cl],
                        start=False, stop=True)
                pss.append(ps)
                ots.append(ot_p.tile([P, d], bf16, tag="ot", bufs=3))
            layernorm_relu_pair(pss, [o[:] for o in ots],
                                "og", "obeta", "o", dst_f32=True)
            for i, st in enumerate((st0, st0 + 1)):
                nc.sync.dma_start(
                    out=out_d.ap()[st * P:(st + 1) * P, :], in_=ots[i][:])

        ot_p.close()
        g_p.close()
        xTbf_p.close()
        xbf_p.close()
        ps_p.close()
        owb_p.close()
        ow8_p.close()
        xT8_p.close()
        pf_p.close()
        tmp_p.close()
        sp.close()
        pp.close()
        # (ps_p is on the separate PSUM stack; SBUF closes above are LIFO)

    nc.compile()
    return nc


def _get_program(c_scale: float, trivial: bool, s: int = S, d: int = D):
    key = (round(float(c_scale), 12), trivial, s, d)
    if key not in _BUILD_CACHE:
        _BUILD_CACHE[key] = _build(c_scale, trivial, s, d)
    return _BUILD_CACHE[key]


def _w_ext(W, dtype):
    """[K, N] weights -> [K, N + DPAD] with col N = MS * row-mean, pad 0."""
    W = np.asarray(W, np.float32)
    k = W.shape[0]
    ext = np.zeros((k, W.shape[1] + DPAD), np.float32)
    ext[:, :W.shape[1]] = W
    ext[:, W.shape[1]] = MS * W.mean(axis=1)
    return np.ascontiguousarray(ext.astype(dtype))


def _prep_in_maps(vision, language, vW, lW, oW, trivial, extras):
    n_b = vision.shape[0]
    vW8 = _w_ext(vW, F8E4)
    lW8 = _w_ext(lW, F8E4)
    oW_ext = _w_ext(oW, np.float32)
    oW_ext[:, D] = 0.0            # final LN mean is computed on device
    oW8 = np.ascontiguousarray(oW_ext.astype(F8E4))
    oWbf = np.ascontiguousarray(oW_ext.astype(BF16))
    in_maps = []
    for b in range(n_b):
        vlT = np.concatenate([vision[b].T, language[b].T], 0)
        m = {
            "vlT8": np.ascontiguousarray(vlT.astype(F8E4)),
            "vW8": vW8, "lW8": lW8, "oW8": oW8, "oWbf": oWbf,
        }
        if not trivial:
            m.update(extras)
        in_maps.append(m)
    return in_maps


def _program_and_inmaps(inputs):
    """(compiled program, per-core input maps) for the given full inputs."""
    vision = np.asarray(inputs["vision_features"], np.float32)
    language = np.asarray(inputs["language_features"], np.float32)
    c_scale = float(np.asarray(inputs["claw"], np.float32).mean()) / TEMPERATURE
    nc = _get_program(c_scale, True)
    in_maps = _prep_in_maps(vision, language, inputs["vW"], inputs["lW"],
                            inputs["oW"], True, {})
    return nc, in_maps


def kernel(vision_features, language_features, vW, vb, vg, vbeta,
           lW, lb, lg, lbeta, claw, oW, ob, og, obeta):
    from concourse import bass_utils

    vision = np.asarray(vision_features, np.float32)
    language = np.asarray(language_features, np.float32)
    c_scale = float(np.asarray(claw, np.float32).mean()) / TEMPERATURE
    # linearized softmax: valid when the logit scale is small (|y| << 1);
    # |sim| <= 1.5*D is a conservative row-norm bound
    assert abs(c_scale) * 1.5 * D < 0.8, "logit scale too large to linearize"

    trivial = (
        np.all(np.asarray(vb) == 0) and np.all(np.asarray(lb) == 0)
        and np.all(np.asarray(ob) == 0)
        and np.all(np.asarray(vg) == 1) and np.all(np.asarray(vbeta) == 0)
        and np.all(np.asarray(lg) == 1) and np.all(np.asarray(lbeta) == 0)
        and np.all(np.asarray(og) == 1) and np.all(np.asarray(obeta) == 0)
    )

    def bias_ext(bv):
        bv = np.asarray(bv, np.float32).reshape(D)
        ext = np.zeros(D + DPAD, np.float32)
        ext[:D] = bv
        ext[D] = MS * bv.mean()
        return ext.reshape(1, D + DPAD).astype(BF16)

    extras = {}
    if not trivial:
        extras = {
            "vb": bias_ext(vb), "lb": bias_ext(lb), "ob": bias_ext(ob),
            "vg": np.asarray(vg, np.float32).reshape(1, D),
            "vbeta": np.asarray(vbeta, np.float32).reshape(1, D),
            "lg": np.asarray(lg, np.float32).reshape(1, D),
            "lbeta": np.asarray(lbeta, np.float32).reshape(1, D),
            "og": np.asarray(og, np.float32).reshape(1, D),
            "obeta": np.asarray(obeta, np.float32).reshape(1, D),
        }

    nc = _get_program(c_scale, trivial)
    in_maps = _prep_in_maps(vision, language, vW, lW, oW, trivial, extras)
    res = bass_utils.run_bass_kernel_spmd(nc, in_maps,
                                          core_ids=list(range(B)))
    return np.stack([np.asarray(res.results[b]["out"], np.float32)
                     for b in range(B)], axis=0)
